# revision 25
# baseline (speedup 1.0000x reference)
"""ContextQueryAttention (BiDAF-style) Trainium2 Bass kernel, v3.

Full inputs -> full output; internally data-parallel over batch across 8
NeuronCores (4 batches per core).

Per-batch math (b dropped; C:[d,t], Q:[d,j], d=512, t=1024, j=128):
  H = C^T, U = Q^T
  S[t,j]  = hbias[t] + ubias[j] + sum_d w_hu[d]*C[d,t]*Q[d,j]
  S_q     = softmax_j(mask(S, mask_Q))         # masked -> -1e30
  S_c     = softmax_t(mask(S, mask_C))
  A       = S_q @ U                            # (t,d)
  q2c     = S_c^T @ H                          # (j,d)
  Bmat    = S_q @ q2c                          # (t,d)
  out     = [H; A; H*A; H*Bmat] as (4d, t)

Implementation notes:
  - All heavy PE matmuls in fp32r (1 cyc/row at free>=256). BIR requires
    fp32r matmul inputs to be written rounded by a compute op, so C gets a
    rounding copy (ctr, on Pool); PSUM->SBUF evacuation copies do the
    rounding for everything else.
  - mask_C applied as a per-partition (t) scalar multiply on the [t,j]
    evacuation of exp(S - MHAT); S_c column sums via tiny matmuls against
    ones. Relies on saturating (non-inf) exp like the fixed-MHAT trick.
  - Merged DMAs (one per input / output block) with 3D "p c t" APs.
  - Queue split: loads + H store on SP, A/HA/HB stores on ACT, so store
    semaphore-waits don't head-of-line block next-batch loads.
  - Software-pipelined emission: head(b+1) [loads, ctr, qw, ub, addc] is
    emitted before body(b), so each engine queue services next-batch head
    work before this batch's tail (HA/HB/stores). Without this, Pool's
    ctr(b+1) sits behind HA(b) and serializes the whole pipeline.
"""

import numpy as np

import concourse.bass as bass
import concourse.tile as tile
from concourse import bacc, mybir
from concourse import bass_utils
from concourse.masks import make_identity

F32 = mybir.dt.float32
F32R = mybir.dt.float32r
I32 = mybir.dt.int32

B, T, J, D = 32, 1024, 128, 512
NCORES = 8
BPC = B // NCORES  # batches per core
MHAT = 100.0  # fixed max-subtraction constant for the softmax over t
NCH = D // 128  # 4 d-chunks
NTC = T // 128  # 8 t-chunks


def _emit_head(nc, pools, consts, aps, b):
    """Loads + input-side prep for batch b. Emitted one batch ahead so these
    sit in front of the previous batch's tail work in every engine queue."""
    (cin, qin, mid, outp, psS_pool, tp_pool, mm_pool, sm) = pools
    (identity, ones128, onescol, w_col, W_H, mqf_all, mcolf, mhat_neg) = consts
    (C, Q, out) = aps

    # ---- loads (SP queue) + early H store ----
    ct = cin.tile([128, NCH * T], F32, tag="ct", name=f"ct{b}")
    nc.sync.dma_start(ct[:].rearrange("p (c t) -> p c t", c=NCH),
                      C[b, :, :].rearrange("(c p) t -> p c t", c=NCH))
    qt = qin.tile([128, NCH * J], F32, tag="qt", name=f"qt{b}")
    nc.sync.dma_start(qt[:].rearrange("p (c j) -> p c j", c=NCH),
                      Q[b, :, :].rearrange("(c p) j -> p c j", c=NCH))
    if b < BPC - 1:
        nc.sync.dma_start(
            out[b, 0:D, :].rearrange("(c p) t -> p c t", c=NCH),
            ct[:].rearrange("p (c t) -> p c t", c=NCH))

    # Pool prep: qw + rounded-C chunks interleaved so the S-core can start
    # on (qw0, ctr0) as soon as possible after the load lands.
    ctr = mid.tile([128, NCH * T], F32R, tag="ctr", bufs=2, name=f"ctr{b}")
    qw = sm.tile([128, NCH * J], F32R, tag="qw", bufs=2, name=f"qw{b}")
    for i in range(NCH):
        nc.gpsimd.tensor_scalar_mul(qw[:, J * i:J * (i + 1)],
                                    qt[:, J * i:J * (i + 1)],
                                    w_col[:, 8 + i:9 + i])
        nc.gpsimd.tensor_copy(ctr[:, T * i:T * (i + 1)],
                              ct[:, T * i:T * (i + 1)])

    return {"ct": ct, "qt": qt, "ctr": ctr, "qw": qw, "b": b}


def _emit_body(nc, pools, consts, aps, st):
    """Main compute + stores for batch b (state from _emit_head)."""
    (cin, qin, mid, outp, psS_pool, tp_pool, mm_pool, sm) = pools
    (identity, ones128, onescol, w_col, W_H, mqf_all, mcolf, mhat_neg) = consts
    (C, Q, out) = aps
    b = st["b"]
    ct, qt, ctr, qw = st["ct"], st["qt"], st["ctr"], st["qw"]

    def ctc(i, sl):  # rounded C chunk i, free-slice sl
        return ctr[:, T * i + sl.start:T * i + sl.stop]

    if b == BPC - 1:
        nc.sync.dma_start(
            out[b, 0:D, :].rearrange("(c p) t -> p c t", c=NCH),
            ct[:].rearrange("p (c t) -> p c t", c=NCH))

    # ---- ubias[j] = sum_d w_u[d] Q[d,j] : N=1 matmuls -> [j,1] ----
    ub_ps = mm_pool.tile([128, 8], F32, tag="mm", name=f"ub{b}")
    for i in range(NCH):
        nc.tensor.matmul(ub_ps[:, 0:1], qt[:, J * i:J * (i + 1)],
                         w_col[:, 4 + i:5 + i],
                         start=(i == 0), stop=(i == NCH - 1))

    # per-batch mask_Q columns: scale = mqf, bias = mqf*ub + (mqf-1)*1e30
    mqf = mqf_all[:, b:b + 1]
    addc = sm.tile([128, 1], F32, tag="addc", name=f"addc{b}")
    nc.vector.tensor_scalar(addc[:], mqf, 1.0e30, -1.0e30,
                            op0=mybir.AluOpType.mult, op1=mybir.AluOpType.add)
    mub = sm.tile([128, 1], F32, tag="mub", name=f"mub{b}")
    nc.vector.tensor_mul(mub[:], mqf, ub_ps[:, 0:1])
    nc.vector.tensor_add(addc[:], addc[:], mub[:])

    # ---- S^T core + hbias into PSUM (fp32r): one bank per t-half ----
    Smq = mid.tile([128, T], F32, tag="smq", bufs=2, name=f"smq{b}")
    ecT = mid.tile([128, T], F32, tag="ect", bufs=2, name=f"ect{b}")
    for h in range(2):
        sl = slice(512 * h, 512 * (h + 1))
        psSh = psS_pool.tile([128, 512], F32, tag="psS", name=f"psS{b}_{h}")
        for i in range(NCH):
            nc.tensor.matmul(psSh[:], qw[:, J * i:J * (i + 1)], ctc(i, sl),
                             start=(i == 0), stop=False)
        for i in range(NCH):
            nc.tensor.matmul(psSh[:], W_H[i][:], ctc(i, sl),
                             start=False, stop=(i == NCH - 1))
        # S_q path: Smq half = psS*mqf + addc  (ACT, PSUM -> SBUF)
        nc.scalar.activation(Smq[:, sl], psSh[:],
                             mybir.ActivationFunctionType.Identity,
                             bias=addc[:], scale=mqf)
        # S_c path: exp with constant max subtraction (saturating exp)
        nc.scalar.activation(ecT[:, sl], psSh[:],
                             mybir.ActivationFunctionType.Exp,
                             bias=mhat_neg[:], scale=1.0)

    # ---- S_q path: transpose Smq -> [t,j] tiles (packed 4 per PSUM bank) ----
    tpq = [tp_pool.tile([128, 512], F32, tag="tp", name=f"tpq{b}_{k}")
           for k in range(2)]
    for c in range(NTC):
        nc.tensor.transpose(tpq[c // 4][:, 128 * (c % 4):128 * (c % 4 + 1)],
                            Smq[:, 128 * c:128 * (c + 1)], identity[:])
    # negated row-max per chunk
    nrmax = sm.tile([128, 8], F32, tag="nrmax", name=f"nrmax{b}")
    for k in range(2):
        nc.vector.tensor_reduce(nrmax[:, 4 * k:4 * (k + 1)],
                                tpq[k][:].rearrange("p (c f) -> p c f", f=128),
                                op=mybir.AluOpType.max,
                                axis=mybir.AxisListType.X, negate=True)
    # exp with per-row bias, fused row-sums; then normalize in place
    e_sb = mid.tile([128, T], F32, tag="smq", bufs=2, name=f"esb{b}")
    esum = sm.tile([128, 8], F32, tag="esum", name=f"esum{b}")
    for c in range(NTC):
        nc.scalar.activation(e_sb[:, 128 * c:128 * (c + 1)],
                             tpq[c // 4][:, 128 * (c % 4):128 * (c % 4 + 1)],
                             mybir.ActivationFunctionType.Exp,
                             bias=nrmax[:, c:c + 1],
                             accum_out=esum[:, c:c + 1])
    resum = sm.tile([128, 8], F32, tag="resum", name=f"resum{b}")
    nc.vector.reciprocal(resum[:], esum[:])
    for c in range(NTC):
        nc.vector.tensor_scalar_mul(e_sb[:, 128 * c:128 * (c + 1)],
                                    e_sb[:, 128 * c:128 * (c + 1)],
                                    resum[:, c:c + 1])
    # transpose back -> S_q^T [j,t] fp32r
    tb = [tp_pool.tile([128, 512], F32, tag="tp", name=f"tb{b}_{k}")
          for k in range(2)]
    for c in range(NTC):
        nc.tensor.transpose(tb[c // 4][:, 128 * (c % 4):128 * (c % 4 + 1)],
                            e_sb[:, 128 * c:128 * (c + 1)], identity[:])
    SqT = mid.tile([128, T], F32R, tag="ect", bufs=2, name=f"sqt{b}")
    for k in range(2):
        nc.scalar.activation(SqT[:, 512 * k:512 * (k + 1)], tb[k][:],
                             mybir.ActivationFunctionType.Identity)

    # ---- e_c transposes -> [t,j] fp32r chunks, mask_C applied per-row ----
    te = [tp_pool.tile([128, 512], F32, tag="tp", name=f"te{b}_{k}")
          for k in range(2)]
    for c in range(NTC):
        nc.tensor.transpose(te[c // 4][:, 128 * (c % 4):128 * (c % 4 + 1)],
                            ecT[:, 128 * c:128 * (c + 1)], identity[:])
    ec_sb = mid.tile([128, T], F32R, tag="ecsb", bufs=1, name=f"ecsb{b}")
    for k in range(2):
        for q in range(4):
            c = 4 * k + q
            nc.vector.tensor_scalar_mul(
                ec_sb[:, 128 * c:128 * (c + 1)],
                te[k][:, 128 * q:128 * (q + 1)],
                mcolf[:, 8 * b + c:8 * b + c + 1])

    # ---- csum[j] = sum_t masked-e_c: tiny matmuls against ones ----
    cs_ps = mm_pool.tile([128, 8], F32, tag="mm", name=f"cs{b}")
    for c in range(NTC):
        nc.tensor.matmul(cs_ps[:, 0:8], ec_sb[:, 128 * c:128 * (c + 1)],
                         onescol[:],
                         start=(c == 0), stop=(c == NTC - 1))
    rc = sm.tile([128, 1], F32, tag="rc", name=f"rc{b}")
    nc.vector.reciprocal(rc[:], cs_ps[:, 0:1])

    # ---- H = C^T tiles [t,d] fp32r (4 transposes per t-chunk) ----
    hsb = mid.tile([128, NTC * 512], F32R, tag="hsb", bufs=1, name=f"hsb{b}")
    for c in range(NTC):
        tH = tp_pool.tile([128, 512], F32, tag="tp", name=f"tH{b}_{c}")
        for i in range(NCH):
            nc.tensor.transpose(tH[:, 128 * i:128 * (i + 1)],
                                ct[:, T * i + 128 * c:T * i + 128 * (c + 1)],
                                identity[:])
        if c % 2 == 0:
            nc.vector.tensor_copy(hsb[:, 512 * c:512 * (c + 1)], tH[:])
        else:
            nc.scalar.activation(hsb[:, 512 * c:512 * (c + 1)], tH[:],
                                 mybir.ActivationFunctionType.Identity)

    # ---- Q^T [j,d] fp32r ----
    tQ = tp_pool.tile([128, 512], F32, tag="tp", name=f"tQ{b}")
    for i in range(NCH):
        nc.tensor.transpose(tQ[:, 128 * i:128 * (i + 1)],
                            qt[:, J * i:J * (i + 1)], identity[:])
    QT = mid.tile([128, 512], F32R, tag="qT", bufs=1, name=f"qT{b}")
    nc.scalar.activation(QT[:], tQ[:],
                         mybir.ActivationFunctionType.Identity)

    # ---- q2c = (1/csum) * sum_c e_c[c].T @ H[c]  -> [j,d] fp32r ----
    psq = mm_pool.tile([128, 512], F32, tag="mm", name=f"psq{b}")
    for c in range(NTC):
        nc.tensor.matmul(psq[:], ec_sb[:, 128 * c:128 * (c + 1)],
                         hsb[:, 512 * c:512 * (c + 1)],
                         start=(c == 0), stop=(c == NTC - 1))
    q2c = mid.tile([128, 512], F32R, tag="q2c", bufs=1, name=f"q2c{b}")
    nc.vector.tensor_scalar_mul(q2c[:], psq[:], rc[:])

    # ---- A^T (fp32r) + H*A; A copies feed Pool early ----
    Am = outp.tile([128, NCH * T], F32, tag="am", bufs=2, name=f"am{b}")
    Ham = outp.tile([128, NCH * T], F32, tag="ham", bufs=1, name=f"ham{b}")
    for m in range(NCH):
        for h in range(2):
            sl = slice(512 * h, 512 * (h + 1))
            psA = mm_pool.tile([128, 512], F32, tag="mm", name=f"psA{b}_{m}{h}")
            nc.tensor.matmul(psA[:], QT[:, 128 * m:128 * (m + 1)], SqT[:, sl],
                             start=True, stop=True)
            eng = nc.vector if h == 0 else nc.scalar
            if h == 0:
                nc.vector.tensor_copy(
                    Am[:, T * m + 512 * h:T * m + 512 * (h + 1)], psA[:])
            else:
                nc.scalar.activation(
                    Am[:, T * m + 512 * h:T * m + 512 * (h + 1)], psA[:],
                    mybir.ActivationFunctionType.Identity)
        nc.sync.dma_start(out[b, D + 128 * m:D + 128 * (m + 1), :],
                          Am[:, T * m:T * (m + 1)])
        eng = nc.gpsimd if m < 3 else nc.vector
        eng.tensor_mul(Ham[:, T * m:T * (m + 1)], ct[:, T * m:T * (m + 1)],
                       Am[:, T * m:T * (m + 1)])
        nc.sync.dma_start(out[b, 2 * D + 128 * m:2 * D + 128 * (m + 1), :],
                          Ham[:, T * m:T * (m + 1)])

    # ---- Bmat^T (fp32r), H*B ----
    Hbm = outp.tile([128, NCH * T], F32, tag="hbm", bufs=1, name=f"hbm{b}")
    for m in range(NCH):
        for h in range(2):
            sl = slice(512 * h, 512 * (h + 1))
            psB = mm_pool.tile([128, 512], F32, tag="mm", name=f"psB{b}_{m}{h}")
            nc.tensor.matmul(psB[:], q2c[:, 128 * m:128 * (m + 1)], SqT[:, sl],
                             start=True, stop=True)
            nc.vector.tensor_mul(Hbm[:, T * m + 512 * h:T * m + 512 * (h + 1)],
                                 ct[:, T * m + 512 * h:T * m + 512 * (h + 1)],
                                 psB[:])
        nc.sync.dma_start(out[b, 3 * D + 128 * m:3 * D + 128 * (m + 1), :],
                          Hbm[:, T * m:T * (m + 1)])




def _build():
    nc = bacc.Bacc("TRN2", target_bir_lowering=False, debug=False,
                   num_devices=NCORES)
    C = nc.dram_tensor("C", [BPC, D, T], F32, kind="ExternalInput").ap()
    Q = nc.dram_tensor("Q", [BPC, D, J], F32, kind="ExternalInput").ap()
    mask_C = nc.dram_tensor("mask_C", [BPC, T], I32, kind="ExternalInput").ap()
    mask_Q = nc.dram_tensor("mask_Q", [BPC, J], I32, kind="ExternalInput").ap()
    weight = nc.dram_tensor("weight", [3 * D], F32, kind="ExternalInput").ap()
    out = nc.dram_tensor("out", [BPC, 4 * D, T], F32,
                         kind="ExternalOutput").ap()

    with tile.TileContext(nc) as tc:
        import contextlib
        with contextlib.ExitStack() as ctx:
            const = ctx.enter_context(tc.tile_pool(name="const", bufs=1))
            cin = ctx.enter_context(tc.tile_pool(name="cin", bufs=3))
            qin = ctx.enter_context(tc.tile_pool(name="qin", bufs=2))
            mid = ctx.enter_context(tc.tile_pool(name="mid", bufs=2))
            outp = ctx.enter_context(tc.tile_pool(name="outp", bufs=2))
            sm = ctx.enter_context(tc.tile_pool(name="sm", bufs=4))
            psS_pool = ctx.enter_context(
                tc.tile_pool(name="psS", bufs=2, space="PSUM"))
            tp_pool = ctx.enter_context(
                tc.tile_pool(name="tp", bufs=3, space="PSUM"))
            mm_pool = ctx.enter_context(
                tc.tile_pool(name="mm", bufs=3, space="PSUM"))

            # ---- constants ----
            identity = const.tile([128, 128], F32, tag="identity")
            make_identity(nc, identity[:])
            ones128 = const.tile([128, 128], F32, tag="ones128")
            nc.gpsimd.memset(ones128[:], 1.0)
            onescol = const.tile([128, 8], F32R, tag="onescol")
            nc.vector.tensor_copy(onescol[:], ones128[:, 0:8])
            # weight -> [128, 12]: cols g*4+c hold weight[g*512 + c*128 + p]
            w_col = const.tile([128, 12], F32, tag="w_col")
            nc.scalar.dma_start(
                w_col[:], weight.rearrange("(g c p) -> p (g c)", p=128, c=4))
            # W_H[i]: w_h chunk broadcast along free dim (rank-1 weights)
            W_H = []
            for i in range(NCH):
                t = const.tile([128, 128], F32R, tag=f"W_H{i}")
                nc.vector.tensor_scalar_mul(t[:], ones128[:], w_col[:, i:i + 1])
                W_H.append(t)
            # mask_C -> [128, BPC*8] fp32: col 8b+c holds mask_C[b, 128c+p]
            mci = const.tile([128, BPC * NTC], I32, tag="mci")
            nc.scalar.dma_start(mci[:],
                                mask_C.rearrange("b (c p) -> p (b c)", p=128))
            mcolf = const.tile([128, BPC * NTC], F32, tag="mcolf")
            nc.vector.tensor_copy(mcolf[:], mci[:])
            # mask_Q -> [128, BPC] fp32
            mqi = const.tile([128, BPC], I32, tag="mqi")
            nc.scalar.dma_start(mqi[:], mask_Q.rearrange("b j -> j b"))
            mqf_all = const.tile([128, BPC], F32, tag="mqf")
            nc.vector.tensor_copy(mqf_all[:], mqi[:])
            mhat_neg = const.tile([128, 1], F32, tag="mhat")
            nc.gpsimd.memset(mhat_neg[:], -MHAT)

            consts = (identity, ones128, onescol, w_col, W_H, mqf_all, mcolf,
                      mhat_neg)
            pools = (cin, qin, mid, outp, psS_pool, tp_pool, mm_pool, sm)
            aps = (C, Q, out)

            # software-pipelined emission: head(b+1) before body(b)
            st = _emit_head(nc, pools, consts, aps, 0)
            for b in range(BPC):
                nxt = (_emit_head(nc, pools, consts, aps, b + 1)
                       if b + 1 < BPC else None)
                _emit_body(nc, pools, consts, aps, st)
                st = nxt

    nc.compile()
    return nc


_NC_CACHE = None


def _get_nc():
    global _NC_CACHE
    if _NC_CACHE is None:
        _NC_CACHE = _build()
    return _NC_CACHE


def kernel(C, Q, mask_C, mask_Q, weight):
    nc = _get_nc()
    C = np.ascontiguousarray(C, dtype=np.float32)
    Q = np.ascontiguousarray(Q, dtype=np.float32)
    mask_C = np.ascontiguousarray(mask_C, dtype=np.int32)
    mask_Q = np.ascontiguousarray(mask_Q, dtype=np.int32)
    weight = np.ascontiguousarray(weight, dtype=np.float32)
    in_maps = []
    for c in range(NCORES):
        sl = slice(BPC * c, BPC * (c + 1))
        in_maps.append({
            "C": C[sl], "Q": Q[sl], "mask_C": mask_C[sl],
            "mask_Q": mask_Q[sl], "weight": weight,
        })
    res = bass_utils.run_bass_kernel_spmd(nc, in_maps,
                                          core_ids=list(range(NCORES)))
    return np.concatenate([res.results[c]["out"] for c in range(NCORES)],
                          axis=0)


# revision 26
# speedup vs baseline: 1.0332x; 1.0332x over previous
"""ContextQueryAttention (BiDAF-style) Trainium2 Bass kernel, v3.

Full inputs -> full output; internally data-parallel over batch across 8
NeuronCores (4 batches per core).

Per-batch math (b dropped; C:[d,t], Q:[d,j], d=512, t=1024, j=128):
  H = C^T, U = Q^T
  S[t,j]  = hbias[t] + ubias[j] + sum_d w_hu[d]*C[d,t]*Q[d,j]
  S_q     = softmax_j(mask(S, mask_Q))         # masked -> -1e30
  S_c     = softmax_t(mask(S, mask_C))
  A       = S_q @ U                            # (t,d)
  q2c     = S_c^T @ H                          # (j,d)
  Bmat    = S_q @ q2c                          # (t,d)
  out     = [H; A; H*A; H*Bmat] as (4d, t)

Implementation notes:
  - All heavy PE matmuls in fp32r (1 cyc/row at free>=256). BIR requires
    fp32r matmul inputs to be written rounded by a compute op, so C gets a
    rounding copy (ctr, on Pool); PSUM->SBUF evacuation copies do the
    rounding for everything else.
  - mask_C applied as a per-partition (t) scalar multiply on the [t,j]
    evacuation of exp(S - MHAT); S_c column sums via tiny matmuls against
    ones. Relies on saturating (non-inf) exp like the fixed-MHAT trick.
  - Merged DMAs (one per input / output block) with 3D "p c t" APs.
  - Queue split: loads + H store on SP, A/HA/HB stores on ACT, so store
    semaphore-waits don't head-of-line block next-batch loads.
  - Software-pipelined emission: head(b+1) [loads, ctr, qw, ub, addc] is
    emitted before body(b), so each engine queue services next-batch head
    work before this batch's tail (HA/HB/stores). Without this, Pool's
    ctr(b+1) sits behind HA(b) and serializes the whole pipeline.
"""

import numpy as np

import concourse.bass as bass
import concourse.tile as tile
from concourse import bacc, mybir
from concourse import bass_utils
from concourse.masks import make_identity

F32 = mybir.dt.float32
F32R = mybir.dt.float32r
I32 = mybir.dt.int32

B, T, J, D = 32, 1024, 128, 512
NCORES = 8
BPC = B // NCORES  # batches per core
MHAT = 100.0  # fixed max-subtraction constant for the softmax over t
NCH = D // 128  # 4 d-chunks
NTC = T // 128  # 8 t-chunks


def _emit_head(nc, pools, consts, aps, b):
    """Loads + input-side prep for batch b. Emitted one batch ahead so these
    sit in front of the previous batch's tail work in every engine queue."""
    (cin, qin, mid, outp, psS_pool, tp_pool, mm_pool, sm) = pools
    (identity, ones128, onescol, w_col, W_H, mqf_all, mcolf, mhat_neg) = consts
    (C, Q, out) = aps

    # ---- loads (SP queue) + early H store ----
    ct = cin.tile([128, NCH * T], F32, tag="ct", name=f"ct{b}")
    nc.sync.dma_start(ct[:].rearrange("p (c t) -> p c t", c=NCH),
                      C[b, :, :].rearrange("(c p) t -> p c t", c=NCH))
    qt = qin.tile([128, NCH * J], F32, tag="qt", name=f"qt{b}")
    nc.sync.dma_start(qt[:].rearrange("p (c j) -> p c j", c=NCH),
                      Q[b, :, :].rearrange("(c p) j -> p c j", c=NCH))
    if b < BPC - 1:
        nc.sync.dma_start(
            out[b, 0:D, :].rearrange("(c p) t -> p c t", c=NCH),
            ct[:].rearrange("p (c t) -> p c t", c=NCH))

    # Pool prep: qw + rounded-C chunks interleaved so the S-core can start
    # on (qw0, ctr0) as soon as possible after the load lands.
    ctr = mid.tile([128, NCH * T], F32R, tag="ctr", bufs=2, name=f"ctr{b}")
    qw = sm.tile([128, NCH * J], F32R, tag="qw", bufs=2, name=f"qw{b}")
    for i in range(NCH):
        nc.gpsimd.tensor_scalar_mul(qw[:, J * i:J * (i + 1)],
                                    qt[:, J * i:J * (i + 1)],
                                    w_col[:, 8 + i:9 + i])
        nc.gpsimd.tensor_copy(ctr[:, T * i:T * (i + 1)],
                              ct[:, T * i:T * (i + 1)])

    return {"ct": ct, "qt": qt, "ctr": ctr, "qw": qw, "b": b}


def _emit_body(nc, pools, consts, aps, st):
    """Main compute + stores for batch b (state from _emit_head)."""
    (cin, qin, mid, outp, psS_pool, tp_pool, mm_pool, sm) = pools
    (identity, ones128, onescol, w_col, W_H, mqf_all, mcolf, mhat_neg) = consts
    (C, Q, out) = aps
    b = st["b"]
    ct, qt, ctr, qw = st["ct"], st["qt"], st["ctr"], st["qw"]

    def ctc(i, sl):  # rounded C chunk i, free-slice sl
        return ctr[:, T * i + sl.start:T * i + sl.stop]

    if b == BPC - 1:
        nc.sync.dma_start(
            out[b, 0:D, :].rearrange("(c p) t -> p c t", c=NCH),
            ct[:].rearrange("p (c t) -> p c t", c=NCH))

    # ---- ubias[j] = sum_d w_u[d] Q[d,j] : N=1 matmuls -> [j,1] ----
    ub_ps = mm_pool.tile([128, 8], F32, tag="mm", name=f"ub{b}")
    for i in range(NCH):
        nc.tensor.matmul(ub_ps[:, 0:1], qt[:, J * i:J * (i + 1)],
                         w_col[:, 4 + i:5 + i],
                         start=(i == 0), stop=(i == NCH - 1))

    # per-batch mask_Q columns: scale = mqf, bias = mqf*ub + (mqf-1)*1e30
    mqf = mqf_all[:, b:b + 1]
    addc = sm.tile([128, 1], F32, tag="addc", name=f"addc{b}")
    nc.vector.tensor_scalar(addc[:], mqf, 1.0e30, -1.0e30,
                            op0=mybir.AluOpType.mult, op1=mybir.AluOpType.add)
    mub = sm.tile([128, 1], F32, tag="mub", name=f"mub{b}")
    nc.vector.tensor_mul(mub[:], mqf, ub_ps[:, 0:1])
    nc.vector.tensor_add(addc[:], addc[:], mub[:])

    # ---- S^T core + hbias into PSUM (fp32r): one bank per t-half ----
    Smq = mid.tile([128, T], F32, tag="smq", bufs=2, name=f"smq{b}")
    ecT = mid.tile([128, T], F32, tag="ect", bufs=2, name=f"ect{b}")
    for h in range(2):
        sl = slice(512 * h, 512 * (h + 1))
        psSh = psS_pool.tile([128, 512], F32, tag="psS", name=f"psS{b}_{h}")
        for i in range(NCH):
            nc.tensor.matmul(psSh[:], qw[:, J * i:J * (i + 1)], ctc(i, sl),
                             start=(i == 0), stop=False)
        for i in range(NCH):
            nc.tensor.matmul(psSh[:], W_H[i][:], ctc(i, sl),
                             start=False, stop=(i == NCH - 1))
        # S_q path: Smq half = psS*mqf + addc  (ACT, PSUM -> SBUF)
        nc.scalar.activation(Smq[:, sl], psSh[:],
                             mybir.ActivationFunctionType.Identity,
                             bias=addc[:], scale=mqf)
        # S_c path: exp with constant max subtraction (saturating exp)
        nc.scalar.activation(ecT[:, sl], psSh[:],
                             mybir.ActivationFunctionType.Exp,
                             bias=mhat_neg[:], scale=1.0)

    # ---- S_q path: transpose Smq -> [t,j] tiles (packed 4 per PSUM bank) ----
    tpq = [tp_pool.tile([128, 512], F32, tag="tp", name=f"tpq{b}_{k}")
           for k in range(2)]
    for c in range(NTC):
        nc.tensor.transpose(tpq[c // 4][:, 128 * (c % 4):128 * (c % 4 + 1)],
                            Smq[:, 128 * c:128 * (c + 1)], identity[:])
    # ---- e_c transposes -> [t,j] fp32r chunks, mask_C applied per-row ----
    te = [tp_pool.tile([128, 512], F32, tag="tp", name=f"te{b}_{k}")
          for k in range(2)]
    for c in range(NTC):
        nc.tensor.transpose(te[c // 4][:, 128 * (c % 4):128 * (c % 4 + 1)],
                            ecT[:, 128 * c:128 * (c + 1)], identity[:])
    # negated row-max per chunk
    nrmax = sm.tile([128, 8], F32, tag="nrmax", name=f"nrmax{b}")
    for k in range(2):
        nc.vector.tensor_reduce(nrmax[:, 4 * k:4 * (k + 1)],
                                tpq[k][:].rearrange("p (c f) -> p c f", f=128),
                                op=mybir.AluOpType.max,
                                axis=mybir.AxisListType.X, negate=True)
    # exp with per-row bias, fused row-sums; then normalize in place
    e_sb = mid.tile([128, T], F32, tag="smq", bufs=2, name=f"esb{b}")
    esum = sm.tile([128, 8], F32, tag="esum", name=f"esum{b}")
    for c in range(NTC):
        nc.scalar.activation(e_sb[:, 128 * c:128 * (c + 1)],
                             tpq[c // 4][:, 128 * (c % 4):128 * (c % 4 + 1)],
                             mybir.ActivationFunctionType.Exp,
                             bias=nrmax[:, c:c + 1],
                             accum_out=esum[:, c:c + 1])
    resum = sm.tile([128, 8], F32, tag="resum", name=f"resum{b}")
    nc.vector.reciprocal(resum[:], esum[:])
    for c in range(NTC):
        nc.vector.tensor_scalar_mul(e_sb[:, 128 * c:128 * (c + 1)],
                                    e_sb[:, 128 * c:128 * (c + 1)],
                                    resum[:, c:c + 1])
    # transpose back -> S_q^T [j,t] fp32r
    tb = [tp_pool.tile([128, 512], F32, tag="tp", name=f"tb{b}_{k}")
          for k in range(2)]
    for c in range(NTC):
        nc.tensor.transpose(tb[c // 4][:, 128 * (c % 4):128 * (c % 4 + 1)],
                            e_sb[:, 128 * c:128 * (c + 1)], identity[:])
    SqT = mid.tile([128, T], F32R, tag="ect", bufs=2, name=f"sqt{b}")
    for k in range(2):
        nc.scalar.activation(SqT[:, 512 * k:512 * (k + 1)], tb[k][:],
                             mybir.ActivationFunctionType.Identity)

    ec_sb = mid.tile([128, T], F32R, tag="ecsb", bufs=1, name=f"ecsb{b}")
    for k in range(2):
        for q in range(4):
            c = 4 * k + q
            nc.vector.tensor_scalar_mul(
                ec_sb[:, 128 * c:128 * (c + 1)],
                te[k][:, 128 * q:128 * (q + 1)],
                mcolf[:, 8 * b + c:8 * b + c + 1])

    # ---- csum[j] = sum_t masked-e_c: tiny matmuls against ones ----
    cs_ps = mm_pool.tile([128, 8], F32, tag="mm", name=f"cs{b}")
    for c in range(NTC):
        nc.tensor.matmul(cs_ps[:, 0:8], ec_sb[:, 128 * c:128 * (c + 1)],
                         onescol[:],
                         start=(c == 0), stop=(c == NTC - 1))
    rc = sm.tile([128, 1], F32, tag="rc", name=f"rc{b}")
    nc.vector.reciprocal(rc[:], cs_ps[:, 0:1])

    # ---- H = C^T tiles [t,d] fp32r (4 transposes per t-chunk) ----
    hsb = mid.tile([128, NTC * 512], F32R, tag="hsb", bufs=1, name=f"hsb{b}")
    for c in range(NTC):
        tH = tp_pool.tile([128, 512], F32, tag="tp", name=f"tH{b}_{c}")
        for i in range(NCH):
            nc.tensor.transpose(tH[:, 128 * i:128 * (i + 1)],
                                ct[:, T * i + 128 * c:T * i + 128 * (c + 1)],
                                identity[:])
        if c % 2 == 0:
            nc.vector.tensor_copy(hsb[:, 512 * c:512 * (c + 1)], tH[:])
        else:
            nc.scalar.activation(hsb[:, 512 * c:512 * (c + 1)], tH[:],
                                 mybir.ActivationFunctionType.Identity)

    # ---- Q^T [j,d] fp32r ----
    tQ = tp_pool.tile([128, 512], F32, tag="tp", name=f"tQ{b}")
    for i in range(NCH):
        nc.tensor.transpose(tQ[:, 128 * i:128 * (i + 1)],
                            qt[:, J * i:J * (i + 1)], identity[:])
    QT = mid.tile([128, 512], F32R, tag="qT", bufs=1, name=f"qT{b}")
    nc.scalar.activation(QT[:], tQ[:],
                         mybir.ActivationFunctionType.Identity)

    # ---- q2c = (1/csum) * sum_c e_c[c].T @ H[c]  -> [j,d] fp32r ----
    psq = mm_pool.tile([128, 512], F32, tag="mm", name=f"psq{b}")
    for c in range(NTC):
        nc.tensor.matmul(psq[:], ec_sb[:, 128 * c:128 * (c + 1)],
                         hsb[:, 512 * c:512 * (c + 1)],
                         start=(c == 0), stop=(c == NTC - 1))
    q2c = mid.tile([128, 512], F32R, tag="q2c", bufs=1, name=f"q2c{b}")
    nc.vector.tensor_scalar_mul(q2c[:], psq[:], rc[:])

    # ---- A^T (fp32r) + H*A; A copies feed Pool early ----
    Am = outp.tile([128, NCH * T], F32, tag="am", bufs=2, name=f"am{b}")
    Ham = outp.tile([128, NCH * T], F32, tag="ham", bufs=1, name=f"ham{b}")
    for m in range(NCH):
        for h in range(2):
            sl = slice(512 * h, 512 * (h + 1))
            psA = mm_pool.tile([128, 512], F32, tag="mm", name=f"psA{b}_{m}{h}")
            nc.tensor.matmul(psA[:], QT[:, 128 * m:128 * (m + 1)], SqT[:, sl],
                             start=True, stop=True)
            eng = nc.vector if h == 0 else nc.scalar
            if h == 0:
                nc.vector.tensor_copy(
                    Am[:, T * m + 512 * h:T * m + 512 * (h + 1)], psA[:])
            else:
                nc.scalar.activation(
                    Am[:, T * m + 512 * h:T * m + 512 * (h + 1)], psA[:],
                    mybir.ActivationFunctionType.Identity)
        nc.sync.dma_start(out[b, D + 128 * m:D + 128 * (m + 1), :],
                          Am[:, T * m:T * (m + 1)])
        eng = nc.gpsimd if m < 3 else nc.vector
        eng.tensor_mul(Ham[:, T * m:T * (m + 1)], ct[:, T * m:T * (m + 1)],
                       Am[:, T * m:T * (m + 1)])
        nc.sync.dma_start(out[b, 2 * D + 128 * m:2 * D + 128 * (m + 1), :],
                          Ham[:, T * m:T * (m + 1)])

    # ---- Bmat^T (fp32r), H*B ----
    Hbm = outp.tile([128, NCH * T], F32, tag="hbm", bufs=1, name=f"hbm{b}")
    for m in range(NCH):
        for h in range(2):
            sl = slice(512 * h, 512 * (h + 1))
            psB = mm_pool.tile([128, 512], F32, tag="mm", name=f"psB{b}_{m}{h}")
            nc.tensor.matmul(psB[:], q2c[:, 128 * m:128 * (m + 1)], SqT[:, sl],
                             start=True, stop=True)
            nc.vector.tensor_mul(Hbm[:, T * m + 512 * h:T * m + 512 * (h + 1)],
                                 ct[:, T * m + 512 * h:T * m + 512 * (h + 1)],
                                 psB[:])
        nc.sync.dma_start(out[b, 3 * D + 128 * m:3 * D + 128 * (m + 1), :],
                          Hbm[:, T * m:T * (m + 1)])




def _build():
    nc = bacc.Bacc("TRN2", target_bir_lowering=False, debug=False,
                   num_devices=NCORES)
    C = nc.dram_tensor("C", [BPC, D, T], F32, kind="ExternalInput").ap()
    Q = nc.dram_tensor("Q", [BPC, D, J], F32, kind="ExternalInput").ap()
    mask_C = nc.dram_tensor("mask_C", [BPC, T], I32, kind="ExternalInput").ap()
    mask_Q = nc.dram_tensor("mask_Q", [BPC, J], I32, kind="ExternalInput").ap()
    weight = nc.dram_tensor("weight", [3 * D], F32, kind="ExternalInput").ap()
    out = nc.dram_tensor("out", [BPC, 4 * D, T], F32,
                         kind="ExternalOutput").ap()

    with tile.TileContext(nc) as tc:
        import contextlib
        with contextlib.ExitStack() as ctx:
            const = ctx.enter_context(tc.tile_pool(name="const", bufs=1))
            cin = ctx.enter_context(tc.tile_pool(name="cin", bufs=3))
            qin = ctx.enter_context(tc.tile_pool(name="qin", bufs=2))
            mid = ctx.enter_context(tc.tile_pool(name="mid", bufs=2))
            outp = ctx.enter_context(tc.tile_pool(name="outp", bufs=2))
            sm = ctx.enter_context(tc.tile_pool(name="sm", bufs=4))
            psS_pool = ctx.enter_context(
                tc.tile_pool(name="psS", bufs=2, space="PSUM"))
            tp_pool = ctx.enter_context(
                tc.tile_pool(name="tp", bufs=3, space="PSUM"))
            mm_pool = ctx.enter_context(
                tc.tile_pool(name="mm", bufs=3, space="PSUM"))

            # ---- constants ----
            identity = const.tile([128, 128], F32, tag="identity")
            make_identity(nc, identity[:])
            ones128 = const.tile([128, 128], F32, tag="ones128")
            nc.gpsimd.memset(ones128[:], 1.0)
            onescol = const.tile([128, 8], F32R, tag="onescol")
            nc.vector.tensor_copy(onescol[:], ones128[:, 0:8])
            # weight -> [128, 12]: cols g*4+c hold weight[g*512 + c*128 + p]
            w_col = const.tile([128, 12], F32, tag="w_col")
            nc.scalar.dma_start(
                w_col[:], weight.rearrange("(g c p) -> p (g c)", p=128, c=4))
            # W_H[i]: w_h chunk broadcast along free dim (rank-1 weights)
            W_H = []
            for i in range(NCH):
                t = const.tile([128, 128], F32R, tag=f"W_H{i}")
                nc.vector.tensor_scalar_mul(t[:], ones128[:], w_col[:, i:i + 1])
                W_H.append(t)
            # mask_C -> [128, BPC*8] fp32: col 8b+c holds mask_C[b, 128c+p]
            mci = const.tile([128, BPC * NTC], I32, tag="mci")
            nc.scalar.dma_start(mci[:],
                                mask_C.rearrange("b (c p) -> p (b c)", p=128))
            mcolf = const.tile([128, BPC * NTC], F32, tag="mcolf")
            nc.vector.tensor_copy(mcolf[:], mci[:])
            # mask_Q -> [128, BPC] fp32
            mqi = const.tile([128, BPC], I32, tag="mqi")
            nc.scalar.dma_start(mqi[:], mask_Q.rearrange("b j -> j b"))
            mqf_all = const.tile([128, BPC], F32, tag="mqf")
            nc.vector.tensor_copy(mqf_all[:], mqi[:])
            mhat_neg = const.tile([128, 1], F32, tag="mhat")
            nc.gpsimd.memset(mhat_neg[:], -MHAT)

            consts = (identity, ones128, onescol, w_col, W_H, mqf_all, mcolf,
                      mhat_neg)
            pools = (cin, qin, mid, outp, psS_pool, tp_pool, mm_pool, sm)
            aps = (C, Q, out)

            # software-pipelined emission: head(b+1) before body(b)
            st = _emit_head(nc, pools, consts, aps, 0)
            for b in range(BPC):
                nxt = (_emit_head(nc, pools, consts, aps, b + 1)
                       if b + 1 < BPC else None)
                _emit_body(nc, pools, consts, aps, st)
                st = nxt

    nc.compile()
    return nc


_NC_CACHE = None


def _get_nc():
    global _NC_CACHE
    if _NC_CACHE is None:
        _NC_CACHE = _build()
    return _NC_CACHE


def kernel(C, Q, mask_C, mask_Q, weight):
    nc = _get_nc()
    C = np.ascontiguousarray(C, dtype=np.float32)
    Q = np.ascontiguousarray(Q, dtype=np.float32)
    mask_C = np.ascontiguousarray(mask_C, dtype=np.int32)
    mask_Q = np.ascontiguousarray(mask_Q, dtype=np.int32)
    weight = np.ascontiguousarray(weight, dtype=np.float32)
    in_maps = []
    for c in range(NCORES):
        sl = slice(BPC * c, BPC * (c + 1))
        in_maps.append({
            "C": C[sl], "Q": Q[sl], "mask_C": mask_C[sl],
            "mask_Q": mask_Q[sl], "weight": weight,
        })
    res = bass_utils.run_bass_kernel_spmd(nc, in_maps,
                                          core_ids=list(range(NCORES)))
    return np.concatenate([res.results[c]["out"] for c in range(NCORES)],
                          axis=0)


# revision 27
# speedup vs baseline: 1.0440x; 1.0104x over previous
"""ContextQueryAttention (BiDAF-style) Trainium2 Bass kernel, v3.

Full inputs -> full output; internally data-parallel over batch across 8
NeuronCores (4 batches per core).

Per-batch math (b dropped; C:[d,t], Q:[d,j], d=512, t=1024, j=128):
  H = C^T, U = Q^T
  S[t,j]  = hbias[t] + ubias[j] + sum_d w_hu[d]*C[d,t]*Q[d,j]
  S_q     = softmax_j(mask(S, mask_Q))         # masked -> -1e30
  S_c     = softmax_t(mask(S, mask_C))
  A       = S_q @ U                            # (t,d)
  q2c     = S_c^T @ H                          # (j,d)
  Bmat    = S_q @ q2c                          # (t,d)
  out     = [H; A; H*A; H*Bmat] as (4d, t)

Implementation notes:
  - All heavy PE matmuls in fp32r (1 cyc/row at free>=256). BIR requires
    fp32r matmul inputs to be written rounded by a compute op, so C gets a
    rounding copy (ctr, on Pool); PSUM->SBUF evacuation copies do the
    rounding for everything else.
  - mask_C applied as a per-partition (t) scalar multiply on the [t,j]
    evacuation of exp(S - MHAT); S_c column sums via tiny matmuls against
    ones. Relies on saturating (non-inf) exp like the fixed-MHAT trick.
  - Merged DMAs (one per input / output block) with 3D "p c t" APs.
  - Queue split: loads + H store on SP, A/HA/HB stores on ACT, so store
    semaphore-waits don't head-of-line block next-batch loads.
  - Software-pipelined emission: head(b+1) [loads, ctr, qw, ub, addc] is
    emitted before body(b), so each engine queue services next-batch head
    work before this batch's tail (HA/HB/stores). Without this, Pool's
    ctr(b+1) sits behind HA(b) and serializes the whole pipeline.
"""

import numpy as np

import concourse.bass as bass
import concourse.tile as tile
from concourse import bacc, mybir
from concourse import bass_utils
from concourse.masks import make_identity

F32 = mybir.dt.float32
F32R = mybir.dt.float32r
I32 = mybir.dt.int32

B, T, J, D = 32, 1024, 128, 512
NCORES = 8
BPC = B // NCORES  # batches per core
MHAT = 100.0  # fixed max-subtraction constant for the softmax over t
NCH = D // 128  # 4 d-chunks
NTC = T // 128  # 8 t-chunks


def _emit_head(nc, pools, consts, aps, b):
    """Loads + input-side prep for batch b. Emitted one batch ahead so these
    sit in front of the previous batch's tail work in every engine queue."""
    (cin, qin, mid, outp, psS_pool, tp_pool, mm_pool, sm) = pools
    (identity, ones128, onescol, w_col, W_H, mqf_all, mcolf, mhat_neg) = consts
    (C, Q, out) = aps

    # ---- loads (SP queue) + early H store ----
    ct = cin.tile([128, NCH * T], F32, tag="ct", name=f"ct{b}")
    nc.sync.dma_start(ct[:].rearrange("p (c t) -> p c t", c=NCH),
                      C[b, :, :].rearrange("(c p) t -> p c t", c=NCH))
    qt = qin.tile([128, NCH * J], F32, tag="qt", name=f"qt{b}")
    nc.sync.dma_start(qt[:].rearrange("p (c j) -> p c j", c=NCH),
                      Q[b, :, :].rearrange("(c p) j -> p c j", c=NCH))
    if b < BPC - 1:
        nc.sync.dma_start(
            out[b, 0:D, :].rearrange("(c p) t -> p c t", c=NCH),
            ct[:].rearrange("p (c t) -> p c t", c=NCH))

    # Pool prep: qw + rounded-C chunks interleaved so the S-core can start
    # on (qw0, ctr0) as soon as possible after the load lands.
    ctr = mid.tile([128, NCH * T], F32R, tag="ctr", bufs=2, name=f"ctr{b}")
    qw = sm.tile([128, NCH * J], F32R, tag="qw", bufs=2, name=f"qw{b}")
    for i in range(NCH):
        nc.gpsimd.tensor_scalar_mul(qw[:, J * i:J * (i + 1)],
                                    qt[:, J * i:J * (i + 1)],
                                    w_col[:, 8 + i:9 + i])
        nc.gpsimd.tensor_copy(ctr[:, T * i:T * (i + 1)],
                              ct[:, T * i:T * (i + 1)])

    return {"ct": ct, "qt": qt, "ctr": ctr, "qw": qw, "b": b}


def _emit_body(nc, pools, consts, aps, st):
    """Main compute + stores for batch b (state from _emit_head)."""
    (cin, qin, mid, outp, psS_pool, tp_pool, mm_pool, sm) = pools
    (identity, ones128, onescol, w_col, W_H, mqf_all, mcolf, mhat_neg) = consts
    (C, Q, out) = aps
    b = st["b"]
    ct, qt, ctr, qw = st["ct"], st["qt"], st["ctr"], st["qw"]

    def ctc(i, sl):  # rounded C chunk i, free-slice sl
        return ctr[:, T * i + sl.start:T * i + sl.stop]

    if b == BPC - 1:
        nc.sync.dma_start(
            out[b, 0:D, :].rearrange("(c p) t -> p c t", c=NCH),
            ct[:].rearrange("p (c t) -> p c t", c=NCH))

    # ---- ubias[j] = sum_d w_u[d] Q[d,j] : N=1 matmuls -> [j,1] ----
    ub_ps = mm_pool.tile([128, 8], F32, tag="mm", name=f"ub{b}")
    for i in range(NCH):
        nc.tensor.matmul(ub_ps[:, 0:1], qt[:, J * i:J * (i + 1)],
                         w_col[:, 4 + i:5 + i],
                         start=(i == 0), stop=(i == NCH - 1))

    # per-batch mask_Q columns: scale = mqf, bias = mqf*ub + (mqf-1)*1e30
    mqf = mqf_all[:, b:b + 1]
    addc = sm.tile([128, 1], F32, tag="addc", name=f"addc{b}")
    nc.vector.tensor_scalar(addc[:], mqf, 1.0e30, -1.0e30,
                            op0=mybir.AluOpType.mult, op1=mybir.AluOpType.add)
    mub = sm.tile([128, 1], F32, tag="mub", name=f"mub{b}")
    nc.vector.tensor_mul(mub[:], mqf, ub_ps[:, 0:1])
    nc.vector.tensor_add(addc[:], addc[:], mub[:])

    # ---- S^T core + hbias into PSUM (fp32r): one bank per t-half ----
    Smq = mid.tile([128, T], F32, tag="smq", bufs=2, name=f"smq{b}")
    ecT = mid.tile([128, T], F32, tag="ect", bufs=2, name=f"ect{b}")
    for h in range(2):
        sl = slice(512 * h, 512 * (h + 1))
        psSh = psS_pool.tile([128, 512], F32, tag="psS", name=f"psS{b}_{h}")
        for i in range(NCH):
            nc.tensor.matmul(psSh[:], qw[:, J * i:J * (i + 1)], ctc(i, sl),
                             start=(i == 0), stop=False)
        for i in range(NCH):
            nc.tensor.matmul(psSh[:], W_H[i][:], ctc(i, sl),
                             start=False, stop=(i == NCH - 1))
        # S_q path: Smq half = psS*mqf + addc  (ACT, PSUM -> SBUF)
        nc.scalar.activation(Smq[:, sl], psSh[:],
                             mybir.ActivationFunctionType.Identity,
                             bias=addc[:], scale=mqf)
        # S_c path: exp with constant max subtraction (saturating exp)
        nc.scalar.activation(ecT[:, sl], psSh[:],
                             mybir.ActivationFunctionType.Exp,
                             bias=mhat_neg[:], scale=1.0)

    # ---- S_q path: transpose Smq -> [t,j] tiles (packed 4 per PSUM bank) ----
    tpq = [tp_pool.tile([128, 512], F32, tag="tp", name=f"tpq{b}_{k}")
           for k in range(2)]
    for c in range(NTC):
        nc.tensor.transpose(tpq[c // 4][:, 128 * (c % 4):128 * (c % 4 + 1)],
                            Smq[:, 128 * c:128 * (c + 1)], identity[:])
    # ---- e_c transposes -> [t,j] fp32r chunks, mask_C applied per-row ----
    te = [tp_pool.tile([128, 512], F32, tag="tp", name=f"te{b}_{k}")
          for k in range(2)]
    for c in range(NTC):
        nc.tensor.transpose(te[c // 4][:, 128 * (c % 4):128 * (c % 4 + 1)],
                            ecT[:, 128 * c:128 * (c + 1)], identity[:])
    # ---- H = C^T tiles [t,d] fp32r (4 transposes per t-chunk) ----
    hsb = mid.tile([128, NTC * 512], F32R, tag="hsb", bufs=1, name=f"hsb{b}")
    for c in range(NTC):
        tH = tp_pool.tile([128, 512], F32, tag="tp", name=f"tH{b}_{c}")
        for i in range(NCH):
            nc.tensor.transpose(tH[:, 128 * i:128 * (i + 1)],
                                ct[:, T * i + 128 * c:T * i + 128 * (c + 1)],
                                identity[:])
        if c % 2 == 0:
            nc.vector.tensor_copy(hsb[:, 512 * c:512 * (c + 1)], tH[:])
        else:
            nc.scalar.activation(hsb[:, 512 * c:512 * (c + 1)], tH[:],
                                 mybir.ActivationFunctionType.Identity)

    # negated row-max per chunk
    nrmax = sm.tile([128, 8], F32, tag="nrmax", name=f"nrmax{b}")
    for k in range(2):
        nc.vector.tensor_reduce(nrmax[:, 4 * k:4 * (k + 1)],
                                tpq[k][:].rearrange("p (c f) -> p c f", f=128),
                                op=mybir.AluOpType.max,
                                axis=mybir.AxisListType.X, negate=True)
    # exp with per-row bias, fused row-sums; then normalize in place
    e_sb = mid.tile([128, T], F32, tag="smq", bufs=2, name=f"esb{b}")
    esum = sm.tile([128, 8], F32, tag="esum", name=f"esum{b}")
    for c in range(NTC):
        nc.scalar.activation(e_sb[:, 128 * c:128 * (c + 1)],
                             tpq[c // 4][:, 128 * (c % 4):128 * (c % 4 + 1)],
                             mybir.ActivationFunctionType.Exp,
                             bias=nrmax[:, c:c + 1],
                             accum_out=esum[:, c:c + 1])
    resum = sm.tile([128, 8], F32, tag="resum", name=f"resum{b}")
    nc.vector.reciprocal(resum[:], esum[:])
    for c in range(NTC):
        nc.vector.tensor_scalar_mul(e_sb[:, 128 * c:128 * (c + 1)],
                                    e_sb[:, 128 * c:128 * (c + 1)],
                                    resum[:, c:c + 1])
    # transpose back -> S_q^T [j,t] fp32r
    tb = [tp_pool.tile([128, 512], F32, tag="tp", name=f"tb{b}_{k}")
          for k in range(2)]
    for c in range(NTC):
        nc.tensor.transpose(tb[c // 4][:, 128 * (c % 4):128 * (c % 4 + 1)],
                            e_sb[:, 128 * c:128 * (c + 1)], identity[:])
    SqT = mid.tile([128, T], F32R, tag="ect", bufs=2, name=f"sqt{b}")
    for k in range(2):
        nc.scalar.activation(SqT[:, 512 * k:512 * (k + 1)], tb[k][:],
                             mybir.ActivationFunctionType.Identity)

    ec_sb = mid.tile([128, T], F32R, tag="ecsb", bufs=1, name=f"ecsb{b}")
    for k in range(2):
        for q in range(4):
            c = 4 * k + q
            nc.vector.tensor_scalar_mul(
                ec_sb[:, 128 * c:128 * (c + 1)],
                te[k][:, 128 * q:128 * (q + 1)],
                mcolf[:, 8 * b + c:8 * b + c + 1])

    # ---- csum[j] = sum_t masked-e_c: tiny matmuls against ones ----
    cs_ps = mm_pool.tile([128, 8], F32, tag="mm", name=f"cs{b}")
    for c in range(NTC):
        nc.tensor.matmul(cs_ps[:, 0:8], ec_sb[:, 128 * c:128 * (c + 1)],
                         onescol[:],
                         start=(c == 0), stop=(c == NTC - 1))
    rc = sm.tile([128, 1], F32, tag="rc", name=f"rc{b}")
    nc.vector.reciprocal(rc[:], cs_ps[:, 0:1])

    # ---- Q^T [j,d] fp32r ----
    tQ = tp_pool.tile([128, 512], F32, tag="tp", name=f"tQ{b}")
    for i in range(NCH):
        nc.tensor.transpose(tQ[:, 128 * i:128 * (i + 1)],
                            qt[:, J * i:J * (i + 1)], identity[:])
    QT = mid.tile([128, 512], F32R, tag="qT", bufs=1, name=f"qT{b}")
    nc.scalar.activation(QT[:], tQ[:],
                         mybir.ActivationFunctionType.Identity)

    # ---- q2c = (1/csum) * sum_c e_c[c].T @ H[c]  -> [j,d] fp32r ----
    psq = mm_pool.tile([128, 512], F32, tag="mm", name=f"psq{b}")
    for c in range(NTC):
        nc.tensor.matmul(psq[:], ec_sb[:, 128 * c:128 * (c + 1)],
                         hsb[:, 512 * c:512 * (c + 1)],
                         start=(c == 0), stop=(c == NTC - 1))
    q2c = mid.tile([128, 512], F32R, tag="q2c", bufs=1, name=f"q2c{b}")
    nc.vector.tensor_scalar_mul(q2c[:], psq[:], rc[:])

    # ---- A^T (fp32r) + H*A; A copies feed Pool early ----
    Am = outp.tile([128, NCH * T], F32, tag="am", bufs=2, name=f"am{b}")
    Ham = outp.tile([128, NCH * T], F32, tag="ham", bufs=1, name=f"ham{b}")
    for m in range(NCH):
        for h in range(2):
            sl = slice(512 * h, 512 * (h + 1))
            psA = mm_pool.tile([128, 512], F32, tag="mm", name=f"psA{b}_{m}{h}")
            nc.tensor.matmul(psA[:], QT[:, 128 * m:128 * (m + 1)], SqT[:, sl],
                             start=True, stop=True)
            eng = nc.vector if h == 0 else nc.scalar
            if h == 0:
                nc.vector.tensor_copy(
                    Am[:, T * m + 512 * h:T * m + 512 * (h + 1)], psA[:])
            else:
                nc.scalar.activation(
                    Am[:, T * m + 512 * h:T * m + 512 * (h + 1)], psA[:],
                    mybir.ActivationFunctionType.Identity)
        nc.sync.dma_start(out[b, D + 128 * m:D + 128 * (m + 1), :],
                          Am[:, T * m:T * (m + 1)])
        eng = nc.gpsimd if m < 3 else nc.vector
        eng.tensor_mul(Ham[:, T * m:T * (m + 1)], ct[:, T * m:T * (m + 1)],
                       Am[:, T * m:T * (m + 1)])
        nc.sync.dma_start(out[b, 2 * D + 128 * m:2 * D + 128 * (m + 1), :],
                          Ham[:, T * m:T * (m + 1)])

    # ---- Bmat^T (fp32r), H*B ----
    Hbm = outp.tile([128, NCH * T], F32, tag="hbm", bufs=1, name=f"hbm{b}")
    for m in range(NCH):
        for h in range(2):
            sl = slice(512 * h, 512 * (h + 1))
            psB = mm_pool.tile([128, 512], F32, tag="mm", name=f"psB{b}_{m}{h}")
            nc.tensor.matmul(psB[:], q2c[:, 128 * m:128 * (m + 1)], SqT[:, sl],
                             start=True, stop=True)
            nc.vector.tensor_mul(Hbm[:, T * m + 512 * h:T * m + 512 * (h + 1)],
                                 ct[:, T * m + 512 * h:T * m + 512 * (h + 1)],
                                 psB[:])
        nc.sync.dma_start(out[b, 3 * D + 128 * m:3 * D + 128 * (m + 1), :],
                          Hbm[:, T * m:T * (m + 1)])




def _build():
    nc = bacc.Bacc("TRN2", target_bir_lowering=False, debug=False,
                   num_devices=NCORES)
    C = nc.dram_tensor("C", [BPC, D, T], F32, kind="ExternalInput").ap()
    Q = nc.dram_tensor("Q", [BPC, D, J], F32, kind="ExternalInput").ap()
    mask_C = nc.dram_tensor("mask_C", [BPC, T], I32, kind="ExternalInput").ap()
    mask_Q = nc.dram_tensor("mask_Q", [BPC, J], I32, kind="ExternalInput").ap()
    weight = nc.dram_tensor("weight", [3 * D], F32, kind="ExternalInput").ap()
    out = nc.dram_tensor("out", [BPC, 4 * D, T], F32,
                         kind="ExternalOutput").ap()

    with tile.TileContext(nc) as tc:
        import contextlib
        with contextlib.ExitStack() as ctx:
            const = ctx.enter_context(tc.tile_pool(name="const", bufs=1))
            cin = ctx.enter_context(tc.tile_pool(name="cin", bufs=3))
            qin = ctx.enter_context(tc.tile_pool(name="qin", bufs=2))
            mid = ctx.enter_context(tc.tile_pool(name="mid", bufs=2))
            outp = ctx.enter_context(tc.tile_pool(name="outp", bufs=2))
            sm = ctx.enter_context(tc.tile_pool(name="sm", bufs=4))
            psS_pool = ctx.enter_context(
                tc.tile_pool(name="psS", bufs=2, space="PSUM"))
            tp_pool = ctx.enter_context(
                tc.tile_pool(name="tp", bufs=3, space="PSUM"))
            mm_pool = ctx.enter_context(
                tc.tile_pool(name="mm", bufs=3, space="PSUM"))

            # ---- constants ----
            identity = const.tile([128, 128], F32, tag="identity")
            make_identity(nc, identity[:])
            ones128 = const.tile([128, 128], F32, tag="ones128")
            nc.gpsimd.memset(ones128[:], 1.0)
            onescol = const.tile([128, 8], F32R, tag="onescol")
            nc.vector.tensor_copy(onescol[:], ones128[:, 0:8])
            # weight -> [128, 12]: cols g*4+c hold weight[g*512 + c*128 + p]
            w_col = const.tile([128, 12], F32, tag="w_col")
            nc.scalar.dma_start(
                w_col[:], weight.rearrange("(g c p) -> p (g c)", p=128, c=4))
            # W_H[i]: w_h chunk broadcast along free dim (rank-1 weights)
            W_H = []
            for i in range(NCH):
                t = const.tile([128, 128], F32R, tag=f"W_H{i}")
                nc.vector.tensor_scalar_mul(t[:], ones128[:], w_col[:, i:i + 1])
                W_H.append(t)
            # mask_C -> [128, BPC*8] fp32: col 8b+c holds mask_C[b, 128c+p]
            mci = const.tile([128, BPC * NTC], I32, tag="mci")
            nc.scalar.dma_start(mci[:],
                                mask_C.rearrange("b (c p) -> p (b c)", p=128))
            mcolf = const.tile([128, BPC * NTC], F32, tag="mcolf")
            nc.vector.tensor_copy(mcolf[:], mci[:])
            # mask_Q -> [128, BPC] fp32
            mqi = const.tile([128, BPC], I32, tag="mqi")
            nc.scalar.dma_start(mqi[:], mask_Q.rearrange("b j -> j b"))
            mqf_all = const.tile([128, BPC], F32, tag="mqf")
            nc.vector.tensor_copy(mqf_all[:], mqi[:])
            mhat_neg = const.tile([128, 1], F32, tag="mhat")
            nc.gpsimd.memset(mhat_neg[:], -MHAT)

            consts = (identity, ones128, onescol, w_col, W_H, mqf_all, mcolf,
                      mhat_neg)
            pools = (cin, qin, mid, outp, psS_pool, tp_pool, mm_pool, sm)
            aps = (C, Q, out)

            # software-pipelined emission: head(b+1) before body(b)
            st = _emit_head(nc, pools, consts, aps, 0)
            for b in range(BPC):
                nxt = (_emit_head(nc, pools, consts, aps, b + 1)
                       if b + 1 < BPC else None)
                _emit_body(nc, pools, consts, aps, st)
                st = nxt

    nc.compile()
    return nc


_NC_CACHE = None


def _get_nc():
    global _NC_CACHE
    if _NC_CACHE is None:
        _NC_CACHE = _build()
    return _NC_CACHE


def kernel(C, Q, mask_C, mask_Q, weight):
    nc = _get_nc()
    C = np.ascontiguousarray(C, dtype=np.float32)
    Q = np.ascontiguousarray(Q, dtype=np.float32)
    mask_C = np.ascontiguousarray(mask_C, dtype=np.int32)
    mask_Q = np.ascontiguousarray(mask_Q, dtype=np.int32)
    weight = np.ascontiguousarray(weight, dtype=np.float32)
    in_maps = []
    for c in range(NCORES):
        sl = slice(BPC * c, BPC * (c + 1))
        in_maps.append({
            "C": C[sl], "Q": Q[sl], "mask_C": mask_C[sl],
            "mask_Q": mask_Q[sl], "weight": weight,
        })
    res = bass_utils.run_bass_kernel_spmd(nc, in_maps,
                                          core_ids=list(range(NCORES)))
    return np.concatenate([res.results[c]["out"] for c in range(NCORES)],
                          axis=0)
